# revision 32
# baseline (speedup 1.0000x reference)
"""Trainium2 Bass kernel for nn_LinearAttention (gated linear attention).

Math (per reference):
    qkv = x @ Wqkv.T ; q,k,v = split(qkv); q,k = elu(.)+1
    per (b,h): running_kv[t]  = d*running_kv[t-1]  + k[t]*v[t]   (elementwise, D=64)
               running_ksum[t]= d*running_ksum[t-1]+ k[t]
    den = clip(sum_d(q*running_ksum), 1e-6); out = q*running_kv/den
    g = sigmoid(out @ Wgate.T + bgate); out = g*out + (1-g)*v
    y = out @ Wout.T

Implementation strategy (8 NeuronCores, SPMD, no collectives):
  - Token-parallel: core c handles batch b=c//2, T-half h=c%2 (2048 tokens)
    plus a 128-token halo before the chunk to warm the decay scan
    (decay=0.95 => truncation error ~0.95^128 ~ 1.4e-3, well under the
    2e-2 gate; measured end-to-end contribution is ~0).  Half 0 gets a
    zero halo + k-mask so its scan state is exactly 0 at t=0.
  - fp8 (e4m3, DoubleRow perf mode => K=256 per matmul) for the q-section,
    k-section and gate projections.  Numpy error analysis: q errors cancel
    in q*ckv/(q*cks); k errors are smoothed by the positive-mean ksum scan;
    gate errors are diluted by sigmoid.  v and Wout stay bf16 (v is
    zero-mean so its errors pass straight through; measured 3.8e-2 if fp8).
  - Everything on-chip lives as [feature(partition), token(free)]; the host
    pre-transposes x (bf16 for the v matmuls + fp8 [128,8,T] DoubleRow
    layout for q/k) and all weights, so no on-chip transpose is needed.
    The final output is produced transposed ([hidden, T]) and un-transposed
    on host.
  - The decay scan runs natively on the Vector engine via
    tensor_tensor_scan (state = d*state + u along the free/time axis),
    chained across token-groups via initial=prev[:, -1:].
  - phi(x)=elu(x)+1 = min(exp(x),1) + relu(x): ACT Exp and ACT Relu read
    the PSUM directly; one DVE scalar_tensor_tensor combines them.
  - den: sum over D=64 partitions via a 0/1 block-diagonal selector matmul
    (PSUM [16,W]); reciprocal_approx_fast; broadcast back to 128 partitions
    via a bf16 selector matmul.
  - bgate rides the Sigmoid drain as the ACT per-partition bias.
"""

import sys

for _p in ('/opt/trn_rl_repo', '/root/.axon_site'):
    if _p not in sys.path:
        sys.path.insert(0, _p)

from contextlib import ExitStack

import ml_dtypes
import numpy as np

import concourse.tile as tile
from concourse import bacc, mybir
from concourse.bass_utils import run_bass_kernel_spmd

F32 = mybir.dt.float32
BF16 = mybir.dt.bfloat16
FP8 = mybir.dt.float8e4
AL = mybir.AluOpType
AF = mybir.ActivationFunctionType
DR = mybir.MatmulPerfMode.DoubleRow

B, T, HID = 4, 4096, 1024
H, D = 16, 64
NK = HID // 128           # 8 hidden (contraction) tiles
ND = NK // 2              # 4 DoubleRow contraction steps
HALF_T = T // 2           # 2048 tokens per core
HALO = 128
TLOC = HALO + HALF_T      # 2176
WG = 512                  # full token-group width
NG = 5                    # groups: [128, 512, 512, 512, 512]
GW = [HALO] + [WG] * 4    # group widths
GS = [0, HALO, HALO + WG, HALO + 2 * WG, HALO + 3 * WG]  # group starts
NH = HID // 128           # 8 tiles per q/k/v section

_cache = {}


def _build_nc():
    nc = bacc.Bacc("TRN2", target_bir_lowering=False, debug=False)

    GPS_JS = ()  # GPSIMD scan offload: rejected by walrus ISA engine check

    xT8 = nc.dram_tensor("xT8", [128, NK, TLOC], FP8, kind="ExternalInput")
    xTb = nc.dram_tensor("xTb", [HID, TLOC], BF16, kind="ExternalInput")
    wqk8 = nc.dram_tensor("wqk8", [128, NK, 2 * HID], FP8, kind="ExternalInput")
    wg8 = nc.dram_tensor("wg8", [128, NK, HID], FP8, kind="ExternalInput")
    wvT = nc.dram_tensor("wvT", [HID, HID], BF16, kind="ExternalInput")
    woutT = nc.dram_tensor("woutT", [HID, HID], BF16, kind="ExternalInput")
    dec_c = nc.dram_tensor("dec_c", [128, NH], F32, kind="ExternalInput")
    mask_c = nc.dram_tensor("mask_c", [128, 1], F32, kind="ExternalInput")
    densel = nc.dram_tensor("densel", [128, NH * H], BF16, kind="ExternalInput")
    bcsel = nc.dram_tensor("bcsel", [H, NH * 128], BF16, kind="ExternalInput")
    bgate_c = nc.dram_tensor("bgate_c", [128, NH], F32, kind="ExternalInput")
    yT = nc.dram_tensor("yT", [HID, HALF_T], BF16, kind="ExternalOutput")

    with tile.TileContext(nc) as tc, ExitStack() as ctx:
        consts = ctx.enter_context(tc.tile_pool(name="consts", bufs=1))
        wq_pool = ctx.enter_context(tc.tile_pool(name="wq", bufs=1))
        wg_pool = ctx.enter_context(tc.tile_pool(name="wgp", bufs=1))
        wo_pool = ctx.enter_context(tc.tile_pool(name="wop", bufs=1))
        xt_pool = ctx.enter_context(tc.tile_pool(name="xt", bufs=2))
        qkv_pool = ctx.enter_context(tc.tile_pool(name="qkv", bufs=9))
        tmp_pool = ctx.enter_context(tc.tile_pool(name="tmp", bufs=2))
        cum_pool = ctx.enter_context(tc.tile_pool(name="cum", bufs=1))
        st_pool = ctx.enter_context(tc.tile_pool(name="st", bufs=2))
        oa_pool = ctx.enter_context(tc.tile_pool(name="oa", bufs=4))
        oa8_pool = ctx.enter_context(tc.tile_pool(name="oa8", bufs=2))
        gt_pool = ctx.enter_context(tc.tile_pool(name="gt", bufs=2))
        mix_pool = ctx.enter_context(tc.tile_pool(name="mix", bufs=8))
        y_pool = ctx.enter_context(tc.tile_pool(name="ysb", bufs=2))
        ps_pool = ctx.enter_context(tc.tile_pool(name="ps", bufs=7, space="PSUM"))
        psd_pool = ctx.enter_context(tc.tile_pool(name="psd", bufs=1, space="PSUM"))

        # small consts first (mask gates the halo k drains)
        dec_s = consts.tile([128, NH], F32, tag="dec")
        nc.gpsimd.dma_start(dec_s[:], dec_c.ap()[:, :])
        mask_s = consts.tile([128, 1], F32, tag="mask")
        nc.gpsimd.dma_start(mask_s[:], mask_c.ap()[:, :])

        # fp8 qkv weights for the q and k sections, DoubleRow layout:
        # [128, NK, od] with dim1 = hid k-subtile; pairs (2k, 2k+1) feed one
        # K=256 DoubleRow matmul.
        wqk_s = wq_pool.tile([128, NK, 2 * HID], FP8, tag="wqk", name="wqk8")

        def load_wqk_sec(sec):  # sec 0 = q, 1 = k
            engs = (nc.sync, nc.scalar, nc.gpsimd)
            for kk in range(NK):
                engs[kk % 3].dma_start(
                    wqk_s[:, kk, HID * sec:HID * (sec + 1)],
                    wqk8.ap()[:, kk, HID * sec:HID * (sec + 1)])

        load_wqk_sec(1)  # k-section: first thing the PE needs

        densel_s = consts.tile([128, NH * H], BF16, tag="densel")
        bcsel_s = consts.tile([H, NH * 128], BF16, tag="bcsel")
        bgate_s = consts.tile([128, NH], F32, tag="bg")
        wg8_s = wg_pool.tile([128, NK, HID], FP8, tag="wg8", name="wg8")
        wv_s, wo_s = [], []
        for k in range(NK):
            wv_s.append(wq_pool.tile([128, HID], BF16, tag=f"wv{k}",
                                     name=f"wv_{k}"))
            wo_s.append(wo_pool.tile([128, HID], BF16, tag=f"wo{k}",
                                     name=f"wo_{k}"))

        def load_v():
            for k in range(NK):
                nc.gpsimd.dma_start(
                    wv_s[k][:], wvT.ap()[128 * k:128 * (k + 1), :])

        def load_rest():
            nc.gpsimd.dma_start(densel_s[:], densel.ap()[:, :])
            nc.gpsimd.dma_start(bcsel_s[:], bcsel.ap()[:, :])
            nc.gpsimd.dma_start(bgate_s[:], bgate_c.ap()[:, :])
            for kk in range(NK):
                nc.gpsimd.dma_start(wg8_s[:, kk, :], wg8.ap()[:, kk, :])
            for k in range(NK):
                nc.gpsimd.dma_start(
                    wo_s[k][:], woutT.ap()[128 * k:128 * (k + 1), :])

        state = {}

        def emit_xt(g):
            w = GW[g]
            tok = slice(GS[g], GS[g] + w)
            x8_t = xt_pool.tile([128, NK, w], FP8, tag="x8", name=f"x8_{g}")
            nc.sync.dma_start(x8_t[:], xT8.ap()[:, :, tok])
            xbs = []
            for k in range(NK):
                xt_t = xt_pool.tile([128, w], BF16, tag="xb", bufs=10,
                                    name=f"xb_{g}_{k}")
                nc.sync.dma_start(xt_t[:], xTb.ap()[128 * k:128 * (k + 1), tok])
                xbs.append(xt_t)
            return x8_t, xbs

        phi_ctr = [0]

        def emit_phi(g, ps, w, out_tile, eng=nc.vector):
            """phi(x) = min(exp(x),1) + relu(x); ACT reads PSUM twice."""
            phi_ctr[0] += 1
            i = phi_ctr[0]
            e = tmp_pool.tile([128, w], BF16, tag="phie", bufs=3,
                              name=f"pe_{i}")
            nc.scalar.activation(e[:], ps[:], AF.Exp)
            r = tmp_pool.tile([128, w], BF16, tag="phir", bufs=3,
                              name=f"pr_{i}")
            nc.scalar.activation(r[:], ps[:], AF.Relu)
            eng.scalar_tensor_tensor(
                out_tile[:], e[:], 1.0, r[:], AL.min, AL.add)

        def emit_qk_sec(g, x8_t, sec, out):
            """fp8 DoubleRow matmuls for the q (sec=0) or k (sec=1) section,
            with the phi drain."""
            w = GW[g]
            is_halo = g == 0
            for ot in range(NH):
                ps = ps_pool.tile([128, w], F32, tag="mm", name=f"qk_{g}_{sec}_{ot}")
                off = HID * sec + 128 * ot
                for kd in range(ND):
                    nc.tensor.matmul(
                        ps[:], wqk_s[:, 2 * kd:2 * kd + 2, off:off + 128],
                        x8_t[:, 2 * kd:2 * kd + 2, :],
                        start=(kd == 0), stop=(kd == ND - 1), perf_mode=DR)
                if sec == 1 and is_halo:
                    kr = tmp_pool.tile([128, w], BF16, tag="kraw", bufs=1,
                                       name=f"kr_{g}_{ot}")
                    emit_phi(g, ps, w, kr)
                    out[ot] = qkv_pool.tile([128, w], BF16, tag="k1", bufs=8,
                                            name=f"k1_{g}_{ot}")
                    nc.vector.tensor_scalar_mul(
                        out[ot][:], kr[:], mask_s[:, 0:1])
                else:
                    tag = "q1" if sec == 0 else "k1"
                    nbufs = 9 if sec == 0 else 8
                    out[ot] = qkv_pool.tile([128, w], BF16, tag=tag,
                                            bufs=nbufs, name=f"{tag}_{g}_{ot}")
                    emit_phi(g, ps, w, out[ot])

        def emit_v_sec(g, xbs, vv):
            w = GW[g]
            for ot in range(NH):
                ps = ps_pool.tile([128, w], F32, tag="mm", name=f"vp_{g}_{ot}")
                for k in range(NK):
                    nc.tensor.matmul(
                        ps[:], wv_s[k][:, 128 * ot:128 * (ot + 1)], xbs[k][:],
                        start=(k == 0), stop=(k == NK - 1))
                vv[ot] = qkv_pool.tile([128, w], BF16, tag="v", bufs=17,
                                       name=f"v_{g}_{ot}")
                nc.scalar.copy(vv[ot][:], ps[:])

        def emit_oa_dl(g, q1, cum_kv, den_ib, vv):
            """qckv mults, bc broadcast matmuls (drained to SBUF so the oa
            multiply hits the DVE 2x path), attention out (fp8 copy for the
            gate), and the (oa - v) delta — after which oa is dead."""
            w = GW[g]
            qckv = [None] * NH
            for j in range(NH):
                qckv[j] = tmp_pool.tile([128, w], BF16, tag="qckv", bufs=4,
                                        name=f"qckv_{g}_{j}")
                nc.vector.tensor_mul(qckv[j][:], q1[j][:], cum_kv[j][:])
            oa8 = oa8_pool.tile([128, NH, w], FP8, tag="oa8", name=f"oa8_{g}")
            dls = [None] * NH
            for j in range(NH):
                bc = ps_pool.tile([128, w], F32, tag="mm", name=f"bc_{g}_{j}")
                nc.tensor.matmul(
                    bc[:], bcsel_s[:, 128 * j:128 * (j + 1)], den_ib[:, :],
                    start=True, stop=True)
                oa = oa_pool.tile([128, w], BF16, tag="oa",
                                  name=f"oa_{g}_{j}")
                nc.vector.tensor_mul(oa[:], qckv[j][:], bc[:])
                nc.scalar.copy(oa8[:, j, :], oa[:])
                dls[j] = tmp_pool.tile([128, w], BF16, tag="dl", bufs=16,
                                       name=f"dl_{g}_{j}")
                nc.gpsimd.tensor_sub(dls[j][:], oa[:], vv[j][:])
            return oa8, dls

        def emit_gate_mix(g, oa8, dls, vv):
            """Gate matmuls fused with the mix so gt tiles die immediately:
            mix = g*oa + (1-g)*v = g*(oa-v) + v = gt*dl + v."""
            w = GW[g]
            mix = [None] * NH
            for ot in range(NH):
                ps = ps_pool.tile([128, w], F32, tag="mm", name=f"gp_{g}_{ot}")
                for kd in range(ND):
                    nc.tensor.matmul(
                        ps[:], wg8_s[:, 2 * kd:2 * kd + 2, 128 * ot:128 * (ot + 1)],
                        oa8[:, 2 * kd:2 * kd + 2, :],
                        start=(kd == 0), stop=(kd == ND - 1), perf_mode=DR)
                gt = gt_pool.tile([128, w], BF16, tag="gt",
                                  name=f"gt_{g}_{ot}")
                nc.scalar.activation(
                    gt[:], ps[:], AF.Sigmoid, bias=bgate_s[:, ot:ot + 1])
                d2 = tmp_pool.tile([128, w], BF16, tag="gd",
                                   name=f"d2_{g}_{ot}")
                nc.vector.tensor_mul(d2[:], gt[:], dls[ot][:])
                mix[ot] = mix_pool.tile([128, w], BF16, tag="mix",
                                        name=f"mix_{g}_{ot}")
                nc.vector.tensor_add(mix[ot][:], d2[:], vv[ot][:])
            return mix

        def emit_y(g, mix):
            w = GW[g]
            out_tok = slice(GS[g] - HALO, GS[g] - HALO + w)
            for ot in range(NH):
                ps = ps_pool.tile([128, w], F32, tag="mm", name=f"yp_{g}_{ot}")
                for k in range(NK):
                    nc.tensor.matmul(
                        ps[:], wo_s[k][:, 128 * ot:128 * (ot + 1)], mix[k][:],
                        start=(k == 0), stop=(k == NK - 1))
                ysb = y_pool.tile([128, w], BF16, tag="ysb",
                                  name=f"ysb_{g}_{ot}")
                nc.scalar.copy(ysb[:], ps[:])
                eng = (nc.sync, nc.scalar)[ot % 2] if g == NG - 1 else nc.sync
                eng.dma_start(
                    yT.ap()[128 * ot:128 * (ot + 1), out_tok], ysb[:])

        def emit_ksum_scans(g, k1, q1):
            """ksum scans + prod tiles: emitted right after the q-section so
            the den chain completes early in the iteration."""
            w = GW[g]
            cum_ks = [None] * NH
            for j in range(NH):
                dec_b = dec_s[:, j:j + 1].broadcast_to([128, w])
                cum_ks[j] = cum_pool.tile([128, w], BF16, tag=f"cks{j}",
                                          name=f"cks_{g}_{j}")
                init_ks = 0.0 if g == 0 else state["ks"][j][:, 0:1]
                eng = nc.gpsimd if j in GPS_JS else nc.vector
                eng.tensor_tensor_scan(
                    cum_ks[j][:], dec_b, k1[j][:], init_ks, AL.mult, AL.add)
            prods = [None] * NH
            if q1[0] is not None:
                for j in range(NH):
                    prods[j] = tmp_pool.tile([128, w], BF16, tag="prod",
                                             bufs=8, name=f"prod_{g}_{j}")
                    nc.vector.tensor_mul(prods[j][:], q1[j][:], cum_ks[j][:])
            nks = [None] * NH
            if g < NG - 1:
                for j in range(NH):
                    nks[j] = st_pool.tile([128, 1], F32, tag=f"sks{j}",
                                          name=f"sks_{g}_{j}")
                    nc.vector.tensor_copy(nks[j][:], cum_ks[j][:, w - 1:w])
            state["ks"] = nks
            return cum_ks, prods

        def emit_kv_scans(g, k1, vv):
            w = GW[g]
            cum_kv = [None] * NH
            kvs = [None] * NH
            for j in range(NH):
                kvs[j] = tmp_pool.tile([128, w], BF16, tag="kvp", bufs=2,
                                       name=f"kv_{g}_{j}")
                nc.gpsimd.tensor_mul(kvs[j][:], k1[j][:], vv[j][:])
            for j in range(NH):
                dec_b = dec_s[:, j:j + 1].broadcast_to([128, w])
                cum_kv[j] = cum_pool.tile([128, w], BF16, tag=f"ckv{j}",
                                          name=f"ckv_{g}_{j}")
                init_kv = 0.0 if g == 0 else state["kv"][j][:, 0:1]
                eng = nc.gpsimd if j in GPS_JS else nc.vector
                eng.tensor_tensor_scan(
                    cum_kv[j][:], dec_b, kvs[j][:], init_kv, AL.mult, AL.add)
            nkv = [None] * NH
            if g < NG - 1:
                for j in range(NH):
                    nkv[j] = st_pool.tile([128, 1], F32, tag=f"skv{j}",
                                          name=f"skv_{g}_{j}")
                    nc.vector.tensor_copy(nkv[j][:], cum_kv[j][:, w - 1:w])
            state["kv"] = nkv
            return cum_kv

        def emit_den(g, prods):
            w = GW[g]
            dps = psd_pool.tile([H, w], F32, tag="den", name=f"dps_{g}")
            for j in range(NH):
                nc.tensor.matmul(
                    dps[:], densel_s[:, H * j:H * (j + 1)], prods[j][:],
                    start=(j == 0), stop=(j == NH - 1))
            den_r = tmp_pool.tile([H, w], F32, tag="denr", name=f"denr_{g}")
            nc.vector.tensor_scalar_max(den_r[:], dps[:], 1e-6)
            den_i = tmp_pool.tile([H, w], F32, tag="deni", name=f"deni_{g}")
            nc.vector.reciprocal_approx_fast(den_i[:], den_r[:])
            den_ib = tmp_pool.tile([H, w], BF16, tag="denib",
                                   name=f"denib_{g}")
            nc.scalar.copy(den_ib[:], den_i[:])
            return den_ib

        # ---- software-pipelined emission --------------------------------
        # One-deep pipeline: group g's oa/dl runs at the END of iteration g
        # (giving the DVE late-iteration work), and only gate/wout/y lag one
        # group (filling the PE's early-iteration slot).  iter i emits:
        #   [xt i][k i][q i][gate+mix i-1][ksum i][v i][wout/y i-1][den i]
        #   [kv scans i][oa/dl i]
        prev = None   # (oa8, dls, vv, g) awaiting gate/mix/wout/y, g = i-1
        for g in range(NG):
            q1 = [None] * NH
            k1 = [None] * NH
            vv = [None] * NH
            x8_t, xbs = emit_xt(g)
            if g == 0:
                load_wqk_sec(0)   # q-section, after xt g0 in queue order
                load_v()          # v weights next
                load_rest()       # one contiguous startup DMA block
            emit_qk_sec(g, x8_t, 1, k1)
            if g > 0:
                emit_qk_sec(g, x8_t, 0, q1)
            if prev is not None:
                mix = emit_gate_mix(prev[3], prev[0], prev[1], prev[2])
            cum_ks, prods = emit_ksum_scans(g, k1, q1)
            emit_v_sec(g, xbs, vv)
            if prev is not None:
                emit_y(prev[3], mix)
                prev = None
            if g > 0:
                den_ib = emit_den(g, prods)
            cum_kv = emit_kv_scans(g, k1, vv)
            if g > 0:
                oa8, dls = emit_oa_dl(g, q1, cum_kv, den_ib, vv)
                prev = (oa8, dls, vv, g)
        # flush: gate/mix/wout/y for the last group
        mix = emit_gate_mix(prev[3], prev[0], prev[1], prev[2])
        emit_y(prev[3], mix)

    nc.compile()
    return nc


def _sigmoid(v):
    return 1.0 / (1.0 + np.exp(-v))


def _fp8(a):
    return np.clip(np.asarray(a, np.float32), -240, 240).astype(
        ml_dtypes.float8_e4m3)


def _make_inputs(x, Wqkv, Wout, Wgate, bgate, decay_param):
    decay = _sigmoid(np.asarray(decay_param, np.float64)).astype(np.float32)
    bf = ml_dtypes.bfloat16
    wqkvT = np.ascontiguousarray(np.asarray(Wqkv, np.float32).T)  # [HID, 3H]
    # fp8 DoubleRow layout [128, NK, od]: element (p, kk, od) = W.T[128kk+p, od]
    wqk8 = np.ascontiguousarray(
        _fp8(wqkvT[:, :2 * HID]).reshape(NK, 128, 2 * HID).transpose(1, 0, 2))
    wg8 = np.ascontiguousarray(
        _fp8(np.asarray(Wgate, np.float32).T).reshape(NK, 128, HID)
        .transpose(1, 0, 2))
    wvT = np.ascontiguousarray(wqkvT[:, 2 * HID:]).astype(bf)
    woutT = np.ascontiguousarray(np.asarray(Wout, np.float32).T).astype(bf)

    p = np.arange(128)
    dec_c = np.empty((128, NH), np.float32)
    for j in range(NH):
        dec_c[:, j] = decay[2 * j + p // 64]
    densel = np.zeros((128, NH * H), np.float32)
    for j in range(NH):
        for pp in range(128):
            densel[pp, H * j + 2 * j + pp // 64] = 1.0
    bcsel = np.zeros((H, NH * 128), np.float32)
    for j in range(NH):
        for m in range(128):
            bcsel[2 * j + m // 64, 128 * j + m] = 1.0
    bgate_c = np.ascontiguousarray(
        np.asarray(bgate, np.float32).reshape(NH, 128).T)

    in_maps = []
    for c in range(8):
        b, half = c // 2, c % 2
        xb = np.asarray(x[b], np.float32)  # [T, HID]
        if half == 0:
            xloc = np.concatenate(
                [np.zeros((HALO, HID), np.float32), xb[:HALF_T]], axis=0)
            mask = np.zeros((128, 1), np.float32)
        else:
            xloc = xb[HALF_T - HALO:]
            mask = np.ones((128, 1), np.float32)
        xT = np.ascontiguousarray(xloc.T)  # [HID, TLOC] f32
        xT8 = np.ascontiguousarray(
            _fp8(xT).reshape(NK, 128, TLOC).transpose(1, 0, 2))
        in_maps.append({
            "xT8": xT8, "xTb": xT.astype(bf),
            "wqk8": wqk8, "wg8": wg8, "wvT": wvT, "woutT": woutT,
            "dec_c": dec_c, "mask_c": mask,
            "densel": densel.astype(bf), "bcsel": bcsel.astype(bf),
            "bgate_c": bgate_c,
        })
    return in_maps


def kernel(x, Wqkv, Wout, Wgate, bgate, decay_param):
    if "nc" not in _cache:
        _cache["nc"] = _build_nc()
    nc = _cache["nc"]
    in_maps = _make_inputs(x, Wqkv, Wout, Wgate, bgate, decay_param)
    res = run_bass_kernel_spmd(nc, in_maps, list(range(8)))
    y = np.empty((B, T, HID), np.float32)
    for c in range(8):
        b, half = c // 2, c % 2
        y[b, half * HALF_T:(half + 1) * HALF_T, :] = res.results[c]["yT"].T
    return y
